# revision 73
# baseline (speedup 1.0000x reference)
"""Trainium2 Bass kernel for nn_Expert_Gate (MMoE: 8 experts, 2 task gates).

Reference computation (all dense, fp32):
    h      = relu(einsum('bi,eih->ebh', x, W1) + b1)          [E, B, H1]
    e_out  = relu(einsum('ebh,eho->ebo', h, W2) + b2)         [E, B, H2]
    gates  = softmax(einsum('bi,tie->tbe', x, Wg) + bg, -1)   [T, B, E]
    towers = einsum('tbe,ebo->tbo', gates, e_out)             [T, B, H2]

Sharding: pure data-parallel over batch. Each of the 8 cores gets B/8 = 2048
rows of x, all weights replicated, no collectives.

Per-core dataflow (Bc = 2048, processed in 4 chunks of 512 rows, each chunk
as 4 b-tiles of 128).  All I/O is fp16 (x/W1/Wg/W2 host-cast, output
host-upcast) to halve HBM traffic and get FWL 2x weight loads; PSUM
accumulation stays fp32 so precision matches the old fp32r path (~7e-4).
  - PE warm-up: ~8 dummy matmuls on a memset scratch tile fill the boot-DMA
    wait so the HAM clock gate releases before real data arrives.
  - L1: h.T chunks [h1, b] = W1[e].T @ xT  (fp16, N=512) -> PSUM, relu+bias
    drained to SBUF fp16 on ACT in [h1, b] layout.
  - gate logits computed transposed (Wg stationary, N=512), exp+bias fused
    into the PSUM->SBUF copy on ACT (fp16), PE-transposed back to [128b, 16];
    softmax normalization on DVE -> gate weights w [128b, (e,t)] fp16.
  - L2: e_out chunks [b, o] = hT_slice.T @ W2[e]  (fp16, N=128) -> stacked
    PSUM [128b, (4e,128o)] halves, relu to SBUF fp16 on ACT.
  - hybrid combine.  Chunks 0..2: since softmax gates are positive,
    g*relu(z) = relu(z)*g, so towers[t] = sum_e eo_e * g_te with g_te a
    per-partition(b) column -- one DVE broadcast multiply + an in-place fp16
    tree sum per (b-tile, task), no PE work, hidden under the next chunk's
    L1.  Last chunk: the PE is idle in the tail while the DVE is not, so it
    uses the diag-matmul combine (towers.T accumulates sum_e eo_e.T @
    diag(g_te) in PSUM) with the diag tiles built on the otherwise-idle
    GpSimd during L1.  All L1 relu drains stay on ACT: putting any on the
    DVE FIFO-blocks them behind combine work and stalls L1 at every chunk
    boundary.
  - software pipelining: gates for chunk c+1 are emitted mid-L1(c); boot DMAs
    are demand-ordered and spread over SP/ACT queues.
  - out is one flat [T, Bc*H2] fp16 tensor: chunks 0..2 b-major from tow_sb,
    the last chunk o-major from outT2 (each combine path gets the
    DMA-friendly layout); host stitches and upcasts.
"""

import sys
from contextlib import ExitStack

import numpy as np

if "/opt/trn_rl_repo" not in sys.path:
    sys.path.append("/opt/trn_rl_repo")

import concourse.bass as bass  # noqa: E402
import concourse.tile as tile  # noqa: E402
from concourse import bacc, mybir  # noqa: E402
from concourse.bass_utils import run_bass_kernel_spmd  # noqa: E402

F32 = mybir.dt.float32
F32R = mybir.dt.float32r
F16 = mybir.dt.float16
AF = mybir.ActivationFunctionType
ALU = mybir.AluOpType

B, I, H1, H2, E, T = 16384, 512, 256, 128, 8, 2
NCORES = 8
BC = B // NCORES          # 2048 rows per core
CHUNK = 512               # rows per pipeline chunk (PSUM free-dim limit)
NCHUNK = BC // CHUNK      # 4
NBT = CHUNK // 128        # 4 b-tiles per chunk
KC_I = I // 128           # 4 contraction chunks for layer 1 / gates
MC_H1 = H1 // 128         # 2 output chunks for layer 1 == K chunks for layer 2

_CACHE: dict = {}


LOAD_STYLE = "coarse"   # "coarse" | "fine"
OUT_STYLE = "half_act"  # "half_act" | "bt_sync"
EO_ALL_ACT = True


def _emit(nc, t, has_b2: bool, reps: int = 1, parts: str = "full",
          loop_loads: bool = False):
    """Emit the per-core program. `t` maps tensor names -> DRAM APs.

    reps>1 wraps the body in a hardware loop (for timing); loop_loads moves
    the input DMAs inside that loop so one iteration == one full inference.
    """
    P = {"gates", "diag", "l1", "l2", "combine"} if parts == "full" else set(
        parts.split(",")
    )
    with tile.TileContext(nc) as tc, ExitStack() as ctx:
        const = ctx.enter_context(tc.tile_pool(name="const", bufs=1))
        ht_p = ctx.enter_context(tc.tile_pool(name="ht", bufs=17))
        eo_p = ctx.enter_context(tc.tile_pool(name="eo", bufs=5))
        seo_p = ctx.enter_context(tc.tile_pool(name="seo", bufs=6))
        dg_p = ctx.enter_context(tc.tile_pool(name="dg", bufs=4))
        sm_p = ctx.enter_context(tc.tile_pool(name="sm", bufs=3))
        # wn lives from gates(c) until the last combine multiply of chunk c,
        # so it gets its own pool -- sharing sm_p would WAR-block the next
        # chunk's exp and head-of-line-stall the ACT queue
        wn_p = ctx.enter_context(tc.tile_pool(name="wn", bufs=3))
        hps_p = ctx.enter_context(tc.tile_pool(name="hps", bufs=3, space="PSUM"))
        eps_p = ctx.enter_context(tc.tile_pool(name="eps", bufs=2, space="PSUM"))
        tps_p = ctx.enter_context(tc.tile_pool(name="tps", bufs=2, space="PSUM"))
        gps_p = ctx.enter_context(tc.tile_pool(name="gps", bufs=1, space="PSUM"))

        # ---- resident SBUF tensors ----
        xt_sb = const.tile([128, KC_I, BC], F16)        # [p, kc, b] 16KB/p
        w1_sb = const.tile([128, E, KC_I, H1], F16)     # lhsT slices [128,128]
        w2_sb = const.tile([128, E, MC_H1, H2], F16)    # rhs slices [128,128]
        wg_sb = const.tile([128, KC_I, E * T], F16)
        # packed small constants: [b1 (16) | ident-f16-as-f32 (64) |
        # id16-f16-as-f32 (8) | bg (1)]
        co_sb = const.tile([128, 89], F32)
        b1_sb = co_sb[:, 0:16]
        id_sb = co_sb[:, 16:80].bitcast(F16)
        id16_sb = co_sb[:16, 80:88].bitcast(F16)
        bg_sb = co_sb[:16, 88:89]
        # towers staging for chunks 0..2: [b-part, t, global-btile, o]
        tow_sb = const.tile([128, T, (NCHUNK - 1) * NBT, H2], F16)
        # towers staging for the last chunk (PE diag combine): [o-part, t, b]
        outT2 = const.tile([128, T, CHUNK], F16)
        if has_b2:
            b2_sb = const.tile([1, E * H2], F16)
            on_sb = const.tile([1, 128], F16)
            nc.sync.dma_start(out=b2_sb[:], in_=t["b2r"])
            nc.sync.dma_start(out=on_sb[:], in_=t["ones1"])

        # x per chunk and W1 per expert-group; first-needed data (x chunk 0
        # kc0, W1 e=0 kc0) lands first.  HWDGE triggers cost ~625ns serial, so
        # keep the DMA count low.
        xt_r = t["xt"].rearrange("(kc p) b -> p kc b", p=128)
        w1_r = t["w1"].rearrange("e (kc p) m -> p e kc m", p=128)

        def load_x(c):
            sl = slice(c * CHUNK, (c + 1) * CHUNK)
            nc.sync.dma_start(out=xt_sb[:, :, sl], in_=xt_r[:, :, sl])

        def load_main():
            # demand order: L1 consumes W1 expert e at ~1.7us*e; xt chunk c at
            # ~14us*c; wg at ~1.7us; consts (b1) at first PSUM drain ~0.9us;
            # w2 at first L2 phase.  Boot DMAs are spread across SP/ACT/DVE
            # queues: each dma_start costs ~600ns on its issuing SEQ plus
            # ~630ns on the shared HWDGE, so fanning out the first few
            # shortens the critical path to the first matmul.
            load_x(0)
            nc.scalar.dma_start(out=w1_sb[:, 0], in_=w1_r[:, 0])
            nc.scalar.dma_start(out=co_sb[:], in_=t["consts"])
            nc.sync.dma_start(out=w1_sb[:, 1], in_=w1_r[:, 1])
            nc.scalar.dma_start(
                out=wg_sb[:], in_=t["wg"].rearrange("(kc p) g -> p kc g", p=128)
            )
            nc.sync.dma_start(out=w1_sb[:, 2:4], in_=w1_r[:, 2:4])
            nc.sync.dma_start(out=w1_sb[:, 4:8], in_=w1_r[:, 4:8])
            load_x(1)
            nc.sync.dma_start(
                out=w2_sb[:],
                in_=t["w2"].rearrange("e (kc p) o -> p e kc o", p=128),
            )
            load_x(2)
            load_x(3)

        def warmup():
            # PE warm-up: the HAM clock gate holds the PE at half clock until
            # ~3.4us of sustained activity.  The first real matmul cannot
            # start before its DMAs land (~3.8us), so spend that idle window
            # on dummy matmuls over a memset scratch tile -- by the time data
            # arrives the PE runs at full clock.  The dummy PSUM bank is
            # reclaimed by the first real accumulation (start=True).
            wu_sb = const.tile([128, 640], F16)
            nc.vector.memset(wu_sb[:], 0.0)
            wu_ps = hps_p.tile([128, CHUNK], F32, name="hp")
            for i in range(8):
                nc.tensor.matmul(
                    wu_ps[:],
                    wu_sb[:, 0:128],
                    wu_sb[:, 128:640],
                    start=True,
                    stop=True,
                )

        if not loop_loads:
            load_main()
            warmup()

        # dummies for part-disabled timing builds
        if "gates" not in P:
            wn_dummy = const.tile([128, NBT * E * T], F16)
            nc.vector.memset(wn_dummy[:], 0.125)
        if "l1" not in P:
            ht_dummy = const.tile([128, MC_H1, CHUNK], F16)
            nc.vector.memset(ht_dummy[:], 0.125)
        if "l2" not in P:
            eo_dummy = const.tile([128, E, H2], F16)
            nc.vector.memset(eo_dummy[:], 0.125)
        if "combine" not in P:
            nc.vector.memset(tow_sb[:], 0.0)
            nc.vector.memset(outT2[:], 0.0)

        def build_diags(wn):
            """Diag tiles for the last chunk's PE combine, built on the
            otherwise-idle GpSimd well before they are needed."""
            dgs = []
            for bt in range(NBT):
                dg = dg_p.tile([128, E * T, 128], F16)
                nc.gpsimd.tensor_mul(
                    dg[:],
                    id_sb[:].unsqueeze(1).broadcast_to([128, E * T, 128]),
                    wn[:, bt * E * T : (bt + 1) * E * T]
                    .unsqueeze(2)
                    .broadcast_to([128, E * T, 128]),
                )
                dgs.append(dg)
            return dgs

        def gates_wn(c):
            """Softmax gate weights for chunk c -> wn [128b, (bt, e, t)]."""
            cs = c * CHUNK
            if "gates" not in P:
                return wn_dummy
            lt_ps = gps_p.tile([16, CHUNK], F32, tag="g")
            for kc in range(KC_I):
                nc.tensor.matmul(
                    lt_ps[:],
                    wg_sb[:, kc, :],
                    xt_sb[:, kc, cs : cs + CHUNK],
                    start=(kc == 0),
                    stop=(kc == KC_I - 1),
                )
            # exp(logits + bg) while leaving PSUM, then transpose back
            ew = sm_p.tile([16, CHUNK], F16)
            nc.scalar.activation(ew[:], lt_ps[:], AF.Exp, bias=bg_sb[:, 0:1])
            gps = gps_p.tile([128, NBT, E * T], F16, tag="g")
            for bt in range(NBT):
                nc.tensor.transpose(
                    gps[:, bt, :],
                    ew[:, bt * 128 : (bt + 1) * 128],
                    id16_sb[:],
                )
            # sum over e (col index = bt*16 + e*2 + t)
            sums = sm_p.tile([128, NBT * T], F32)
            nc.vector.reduce_sum(
                sums[:].rearrange("p (bt t) -> p bt t", t=T),
                gps[:].rearrange("p bt (e t) -> p bt t e", e=E, t=T),
                axis=mybir.AxisListType.X,
            )
            recip = sm_p.tile([128, NBT * T], F32)
            nc.vector.reciprocal(recip[:], sums[:])
            wn = wn_p.tile([128, NBT * E * T], F16)
            nc.vector.tensor_mul(
                wn[:].rearrange("p (bt e t) -> p bt e t", e=E, t=T),
                gps[:].rearrange("p bt (e t) -> p bt e t", e=E, t=T),
                recip[:]
                .rearrange("p (bt t) -> p bt t", t=T)
                .unsqueeze(2)
                .broadcast_to([128, NBT, E, T]),
            )
            return wn

        rep_ctx = tc.For_i(0, reps, 1) if reps > 1 else None
        if rep_ctx is not None:
            ctx.enter_context(rep_ctx)
        if loop_loads:
            load_main()
        pending: dict = {}

        def emit_l1(c):
            """Layer 1 + relu for chunk c; chunk c+1's gates+diag are emitted
            mid-phase so their PE/ACT/DVE ops hide under L1."""
            cs = c * CHUNK
            hts = []
            for e in range(E):
                if "l1" in P:
                    ht = ht_p.tile([128, MC_H1, CHUNK], F16)
                    for mc in range(MC_H1):
                        hp = hps_p.tile([128, CHUNK], F32)
                        for kc in range(KC_I):
                            nc.tensor.matmul(
                                hp[:],
                                w1_sb[:, e, kc, mc * 128 : (mc + 1) * 128],
                                xt_sb[:, kc, cs : cs + CHUNK],
                                start=(kc == 0),
                                stop=(kc == KC_I - 1),
                            )
                        bcol = b1_sb[:, e * MC_H1 + mc : e * MC_H1 + mc + 1]
                        # all L1 drains on ACT: L1 then never waits on the
                        # DVE queue, which carries the gate-combine work
                        nc.scalar.activation(
                            ht[:, mc, :], hp[:], AF.Relu, bias=bcol
                        )
                    hts.append(ht)
                else:
                    hts.append(ht_dummy)
                if c == 0 and e == 0:
                    pending[0] = gates_wn(0)
            if c + 1 < NCHUNK:
                pending[c + 1] = gates_wn(c + 1)
                if c + 1 == NCHUNK - 1 and "combine" in P:
                    pending["dgs"] = build_diags(pending[c + 1])
            return hts

        def emit_l2(c, bt, hts):
            """Layer 2 + relu for b-tile bt of chunk c -> eo tile (fp16)."""
            bsl = slice(bt * 128, (bt + 1) * 128)
            if "l2" not in P:
                return eo_dummy
            eo = eo_p.tile([128, E, H2], F16)
            # two half-stacks: the relu drain of half 0 runs under half 1's
            # matmuls
            for half in range(2):
                eps = eps_p.tile([128, E // 2, H2], F32)
                for ei in range(E // 2):
                    e = half * (E // 2) + ei
                    for kc in range(MC_H1):
                        nc.tensor.matmul(
                            eps[:, ei, :],
                            hts[e][:, kc, bsl],
                            w2_sb[:, e, kc, :],
                            start=(kc == 0),
                            stop=(kc == MC_H1 - 1 and not has_b2),
                        )
                    if has_b2:
                        nc.tensor.matmul(
                            eps[:, ei, :],
                            on_sb[:1, :],
                            b2_sb[:1, e * H2 : (e + 1) * H2],
                            start=False,
                            stop=True,
                        )
                eo_sl = eo[:, half * (E // 2) : (half + 1) * (E // 2), :]
                if (bt + half) % 2 == 0 or EO_ALL_ACT:
                    nc.scalar.activation(
                        eo_sl.rearrange("p e o -> p (e o)"),
                        eps[:].rearrange("p e o -> p (e o)"),
                        AF.Relu,
                    )
                else:
                    nc.vector.tensor_scalar_max(
                        eo_sl.rearrange("p e o -> p (e o)"),
                        eps[:].rearrange("p e o -> p (e o)"),
                        0.0,
                    )
            return eo

        # flat output: [T, BC3*H2 (chunks 0..2, b-major) | H2*CHUNK (chunk 3,
        # o-major)] -- the two combine paths produce transposed layouts and
        # each gets the DMA-friendly one; the host stitches them.
        BC3 = (NCHUNK - 1) * CHUNK
        out1_r = t["out"][:, 0 : BC3 * H2].rearrange(
            "t (g p o) -> p t g o", p=128, o=H2
        )
        out2_r = t["out"][:, BC3 * H2 :].rearrange("t (o b) -> o t b", o=H2)

        def emit_combine(c, bt, eo, wn, dgs):
            """Gate combine for b-tile bt of chunk c.

            Chunks 0..2: towers[t] = sum_e eo_e * g_te on the DVE (g_te is a
            per-partition column; valid because g>0 commutes with the relu
            already applied to eo).  This runs off the PE and hides under the
            next chunk's L1.
            Last chunk: PE diag-matmul combine (towers.T accumulates
            sum_e eo_e.T @ diag(g_te) in PSUM) -- the PE is idle in the tail
            while the DVE is not, and the diag tiles were built on GpSimd
            long before.
            """
            if "combine" not in P:
                return
            if c < NCHUNK - 1:
                g = c * NBT + bt
                wn_v = wn[:].rearrange("p (bt e t) -> p bt e t", e=E, t=T)
                for t_ in range(T):
                    seo = seo_p.tile([128, E, H2], F16)
                    nc.vector.tensor_mul(
                        seo[:],
                        eo[:],
                        wn_v[:, bt, :, t_ : t_ + 1].broadcast_to([128, E, H2]),
                    )
                    # summing 8 fp16 terms into fp16 adds ~2^-11 relative
                    # error, well inside the 2e-2 budget
                    with nc.allow_low_precision(reason="fp16 gate combine"):
                        nc.vector.tensor_add(
                            seo[:, 0:4, :], seo[:, 0:4, :], seo[:, 4:8, :]
                        )
                        nc.vector.tensor_add(
                            seo[:, 0:2, :], seo[:, 0:2, :], seo[:, 2:4, :]
                        )
                        nc.vector.tensor_add(
                            tow_sb[:, t_, g, :], seo[:, 0, :], seo[:, 1, :]
                        )
                if bt == NBT - 1:
                    # out-DMA per chunk, one per task (DMA APs are limited to
                    # 3 dims)
                    gs = slice(c * NBT, (c + 1) * NBT)
                    for t_ in range(T):
                        nc.sync.dma_start(
                            out=out1_r[:, t_, gs, :], in_=tow_sb[:, t_, gs, :]
                        )
            else:
                tps = tps_p.tile([128, T, 128], F32)
                tflat = tps[:].rearrange("p t b -> p (t b)")
                for e in range(E):
                    nc.tensor.matmul(
                        tflat,
                        eo[:, e, :],
                        dgs[bt][:, e * T : (e + 1) * T, :].rearrange(
                            "p t b -> p (t b)"
                        ),
                        start=(e == 0),
                        stop=(e == E - 1),
                    )
                dst = outT2[:, :, bt * 128 : (bt + 1) * 128]
                if bt % 2 == 0:
                    nc.vector.tensor_copy(dst, tps[:])
                else:
                    nc.scalar.copy(dst, tps[:])
                if bt % 2 == 1:
                    # out-DMA per half-chunk; the final one goes on the idle
                    # SP queue (shorter tail)
                    bs = slice((bt - 1) * 128, (bt + 1) * 128)
                    eng = nc.sync if bt == NBT - 1 else nc.scalar
                    eng.dma_start(out=out2_r[:, :, bs], in_=outT2[:, :, bs])

        # per-chunk: L1, then L2 per b-tile; the gate-scaled combine runs on
        # the vector engines and is off the PE critical path entirely.
        for c in range(NCHUNK):
            hts = emit_l1(c)
            wn = pending.pop(c)
            if c < NCHUNK - 1:
                for bt in range(NBT):
                    emit_combine(c, bt, emit_l2(c, bt, hts), wn, None)
            else:
                # last chunk: PE combine is software-pipelined one b-tile
                # deep so its wait on the eo relu hides under L2 matmuls
                dgs = pending.pop("dgs", None)
                eos = {}
                for bt in range(NBT):
                    eos[bt] = emit_l2(c, bt, hts)
                    if bt >= 1:
                        emit_combine(c, bt - 1, eos.pop(bt - 1), wn, dgs)
                emit_combine(c, NBT - 1, eos.pop(NBT - 1), wn, dgs)


def _build(has_b2: bool, reps: int = 1, parts: str = "full"):
    nc = bacc.Bacc("TRN2", target_bir_lowering=False, debug=False)
    t = {
        "xt": nc.dram_tensor("xt", [I, BC], F16, kind="ExternalInput").ap(),
        "w1": nc.dram_tensor("w1", [E, I, H1], F16, kind="ExternalInput").ap(),
        "w2": nc.dram_tensor("w2", [E, H1, H2], F16, kind="ExternalInput").ap(),
        "wg": nc.dram_tensor("wg", [I, E * T], F16, kind="ExternalInput").ap(),
        "consts": nc.dram_tensor("consts", [128, 89], F32, kind="ExternalInput").ap(),
        "out": nc.dram_tensor("out", [T, BC * H2], F16, kind="ExternalOutput").ap(),
    }
    if has_b2:
        t["b2r"] = nc.dram_tensor("b2r", [1, E * H2], F16, kind="ExternalInput").ap()
        t["ones1"] = nc.dram_tensor("ones1", [1, 128], F16, kind="ExternalInput").ap()
    _emit(nc, t, has_b2, reps=reps, parts=parts)
    nc.compile()
    return nc


def _get_nc(has_b2: bool):
    key = ("nc", has_b2)
    if key not in _CACHE:
        _CACHE[key] = _build(has_b2)
    return _CACHE[key]


def _host_consts(b1=None, bg=None):
    """Packed consts: [b1 (16) | ident-f16 (64) | id16-f16 (8) | bg (1)]."""
    coh = np.zeros((128, 89), np.float32)
    if b1 is not None:
        coh[:, 0:16] = np.broadcast_to(
            b1.reshape(E, MC_H1, 128).transpose(2, 0, 1), (128, E, MC_H1)
        ).reshape(128, E * MC_H1)
    coh[:, 16:80] = np.ascontiguousarray(np.eye(128, dtype=np.float16)).view(
        np.float32
    )
    coh[:16, 80:88] = np.ascontiguousarray(np.eye(16, dtype=np.float16)).view(
        np.float32
    )
    if bg is not None:
        coh[:16, 88] = bg.T.reshape(E * T)
    return coh


def _host_prep(x, W1, b1, W2, b2, Wg, bg, has_b2):
    w1h = np.ascontiguousarray(W1.astype(np.float16))
    w2h = np.ascontiguousarray(W2.astype(np.float16))
    wgh = np.ascontiguousarray(
        Wg.transpose(1, 2, 0).reshape(I, E * T).astype(np.float16)
    )
    xth = np.ascontiguousarray(x.T.astype(np.float16))
    coh = _host_consts(b1, bg)
    in_maps = []
    for core in range(NCORES):
        m = {
            "xt": np.ascontiguousarray(xth[:, core * BC : (core + 1) * BC]),
            "w1": w1h,
            "w2": w2h,
            "wg": wgh,
            "consts": coh,
        }
        if has_b2:
            m["b2r"] = b2.astype(np.float16).reshape(1, E * H2)
            m["ones1"] = np.ones((1, 128), np.float16)
        in_maps.append(m)
    return in_maps


def _build_runner(nc):
    """Cached replica of bass2jax.run_bass_via_pjrt's multi-core path: the
    jitted shard_map callable is built once and reused across kernel() calls."""
    import jax
    from jax.experimental.shard_map import shard_map
    from jax.sharding import Mesh, PartitionSpec

    from concourse import bass2jax, mybir as mb

    bass2jax.install_neuronx_cc_hook()
    partition_name = (
        nc.partition_id_tensor.name if nc.partition_id_tensor else None
    )
    in_names, out_names, out_avals, zero_shapes = [], [], [], []
    for alloc in nc.m.functions[0].allocations:
        if not isinstance(mb.MemoryLocationSet, type) or not isinstance(
            alloc, mb.MemoryLocationSet
        ):
            continue
        name = alloc.memorylocations[0].name
        if alloc.kind == "ExternalInput":
            if name != partition_name:
                in_names.append(name)
        elif alloc.kind == "ExternalOutput":
            shape = tuple(alloc.tensor_shape)
            dtype = mb.dt.np(alloc.dtype)
            out_names.append(name)
            out_avals.append(jax.core.ShapedArray(shape, dtype))
            zero_shapes.append((shape, dtype))
    n_params = len(in_names)
    n_outs = len(out_avals)
    all_in_names = list(in_names) + list(out_names)
    if partition_name is not None:
        all_in_names.append(partition_name)
    donate = tuple(range(n_params, n_params + n_outs))

    def _body(*args):
        operands = list(args)
        if partition_name is not None:
            operands.append(bass2jax.partition_id_tensor())
        outs = bass2jax._bass_exec_p.bind(
            *operands,
            out_avals=tuple(out_avals),
            in_names=tuple(all_in_names),
            out_names=tuple(out_names),
            lowering_input_output_aliases=(),
            sim_require_finite=True,
            sim_require_nnan=True,
            nc=nc,
        )
        return tuple(outs)

    devices = jax.devices()[:NCORES]
    mesh = Mesh(np.asarray(devices), ("core",))
    in_specs = (PartitionSpec("core"),) * (n_params + n_outs)
    out_specs = (PartitionSpec("core"),) * n_outs
    sharded = jax.jit(
        shard_map(
            _body, mesh=mesh, in_specs=in_specs, out_specs=out_specs,
            check_rep=False,
        ),
        donate_argnums=donate,
        keep_unused=True,
    )

    def run(in_maps):
        concat_in = [
            np.concatenate([np.asarray(m[name]) for m in in_maps], axis=0)
            for name in in_names
        ]
        concat_zeros = [
            np.zeros((NCORES * s[0], *s[1:]), d) for s, d in zero_shapes
        ]
        out_arrs = sharded(*concat_in, *concat_zeros)
        return [
            {
                name: np.asarray(out_arrs[i]).reshape(
                    NCORES, *zero_shapes[i][0]
                )[c]
                for i, name in enumerate(out_names)
            }
            for c in range(NCORES)
        ]

    return run


def kernel(x, W1, b1, W2, b2, Wg, bg):
    x = np.asarray(x, np.float32)
    W1 = np.asarray(W1, np.float32)
    b1 = np.asarray(b1, np.float32)
    W2 = np.asarray(W2, np.float32)
    b2 = np.asarray(b2, np.float32)
    Wg = np.asarray(Wg, np.float32)
    bg = np.asarray(bg, np.float32)

    has_b2 = bool(np.any(b2))
    nc = _get_nc(has_b2)
    in_maps = _host_prep(x, W1, b1, W2, b2, Wg, bg, has_b2)

    key = ("runner", has_b2)
    try:
        if key not in _CACHE:
            _CACHE[key] = _build_runner(nc)
        results = _CACHE[key](in_maps)
    except Exception:
        _CACHE.pop(key, None)
        results = run_bass_kernel_spmd(
            nc, in_maps, core_ids=list(range(NCORES))
        ).results
    # gather: per-core flat out [T, BC*H2] fp16 = [chunks 0-2 b-major |
    # chunk 3 o-major] -> [T, B, H2] fp32
    bc3 = (NCHUNK - 1) * CHUNK
    outs = []
    for r in results:
        flat = r["out"]
        p1 = flat[:, : bc3 * H2].reshape(T, bc3, H2)
        p2 = flat[:, bc3 * H2 :].reshape(T, H2, CHUNK).transpose(0, 2, 1)
        outs.append(np.concatenate([p1, p2], axis=1))
    return np.concatenate(outs, axis=1).astype(np.float32)



# revision 75
# speedup vs baseline: 1.2371x; 1.2371x over previous
"""Trainium2 Bass kernel for nn_Expert_Gate (MMoE: 8 experts, 2 task gates).

Reference computation (all dense, fp32):
    h      = relu(einsum('bi,eih->ebh', x, W1) + b1)          [E, B, H1]
    e_out  = relu(einsum('ebh,eho->ebo', h, W2) + b2)         [E, B, H2]
    gates  = softmax(einsum('bi,tie->tbe', x, Wg) + bg, -1)   [T, B, E]
    towers = einsum('tbe,ebo->tbo', gates, e_out)             [T, B, H2]

Sharding: pure data-parallel over batch. Each of the 8 cores gets B/8 = 2048
rows of x, all weights replicated, no collectives.

Per-core dataflow (Bc = 2048, processed in 4 chunks of 512 rows, each chunk
as 4 b-tiles of 128).  All I/O is fp16 (x/W1/Wg/W2 host-cast, output
host-upcast) to halve HBM traffic and get FWL 2x weight loads; PSUM
accumulation stays fp32 so precision matches the old fp32r path (~7e-4).
  - PE warm-up: ~8 dummy matmuls on a memset scratch tile fill the boot-DMA
    wait so the HAM clock gate releases before real data arrives.
  - L1: h.T chunks [h1, b] = W1[e].T @ xT  (fp16, N=512) -> PSUM, relu+bias
    drained to SBUF fp16 on ACT in [h1, b] layout.
  - gate logits computed transposed (Wg stationary, N=512), exp+bias fused
    into the PSUM->SBUF copy on ACT (fp16), PE-transposed back to [128b, 16];
    softmax normalization on DVE -> gate weights w [128b, (e,t)] fp16.
  - L2: e_out chunks [b, o] = hT_slice.T @ W2[e]  (fp16, N=128) -> stacked
    PSUM [128b, (4e,128o)] halves, relu to SBUF fp16 on ACT.
  - hybrid combine.  Chunks 0..2: since softmax gates are positive,
    g*relu(z) = relu(z)*g, so towers[t] = sum_e eo_e * g_te with g_te a
    per-partition(b) column -- one DVE broadcast multiply + an in-place fp16
    tree sum per (b-tile, task), no PE work, hidden under the next chunk's
    L1.  Last chunk: the PE is idle in the tail while the DVE is not, so it
    uses the diag-matmul combine (towers.T accumulates sum_e eo_e.T @
    diag(g_te) in PSUM) with the diag tiles built on the otherwise-idle
    GpSimd during L1.  All L1 relu drains stay on ACT: putting any on the
    DVE FIFO-blocks them behind combine work and stalls L1 at every chunk
    boundary.
  - software pipelining: gates for chunk c+1 are emitted mid-L1(c); boot DMAs
    are demand-ordered and spread over SP/ACT queues.
  - out is one flat [T, Bc*H2] fp16 tensor: chunks 0..2 b-major from tow_sb,
    the last chunk o-major from outT2 (each combine path gets the
    DMA-friendly layout); host stitches and upcasts.
"""

import sys
from contextlib import ExitStack

import numpy as np

if "/opt/trn_rl_repo" not in sys.path:
    sys.path.append("/opt/trn_rl_repo")

import concourse.bass as bass  # noqa: E402
import concourse.tile as tile  # noqa: E402
from concourse import bacc, mybir  # noqa: E402
from concourse.bass_utils import run_bass_kernel_spmd  # noqa: E402

F32 = mybir.dt.float32
F32R = mybir.dt.float32r
F16 = mybir.dt.float16
AF = mybir.ActivationFunctionType
ALU = mybir.AluOpType

B, I, H1, H2, E, T = 16384, 512, 256, 128, 8, 2
NCORES = 8
BC = B // NCORES          # 2048 rows per core
CHUNK = 512               # rows per pipeline chunk (PSUM free-dim limit)
NCHUNK = BC // CHUNK      # 4
NBT = CHUNK // 128        # 4 b-tiles per chunk
KC_I = I // 128           # 4 contraction chunks for layer 1 / gates
MC_H1 = H1 // 128         # 2 output chunks for layer 1 == K chunks for layer 2

_CACHE: dict = {}


LOAD_STYLE = "coarse"   # "coarse" | "fine"
OUT_STYLE = "half_act"  # "half_act" | "bt_sync"
EO_ALL_ACT = True


def _emit(nc, t, has_b2: bool, reps: int = 1, parts: str = "full",
          loop_loads: bool = False):
    """Emit the per-core program. `t` maps tensor names -> DRAM APs.

    reps>1 wraps the body in a hardware loop (for timing); loop_loads moves
    the input DMAs inside that loop so one iteration == one full inference.
    """
    P = {"gates", "diag", "l1", "l2", "combine"} if parts == "full" else set(
        parts.split(",")
    )
    with tile.TileContext(nc) as tc, ExitStack() as ctx:
        const = ctx.enter_context(tc.tile_pool(name="const", bufs=1))
        ht_p = ctx.enter_context(tc.tile_pool(name="ht", bufs=17))
        eo_p = ctx.enter_context(tc.tile_pool(name="eo", bufs=5))
        seo_p = ctx.enter_context(tc.tile_pool(name="seo", bufs=6))
        dg_p = ctx.enter_context(tc.tile_pool(name="dg", bufs=4))
        sm_p = ctx.enter_context(tc.tile_pool(name="sm", bufs=3))
        # wn lives from gates(c) until the last combine multiply of chunk c,
        # so it gets its own pool -- sharing sm_p would WAR-block the next
        # chunk's exp and head-of-line-stall the ACT queue
        wn_p = ctx.enter_context(tc.tile_pool(name="wn", bufs=3))
        hps_p = ctx.enter_context(tc.tile_pool(name="hps", bufs=3, space="PSUM"))
        eps_p = ctx.enter_context(tc.tile_pool(name="eps", bufs=2, space="PSUM"))
        tps_p = ctx.enter_context(tc.tile_pool(name="tps", bufs=2, space="PSUM"))
        gps_p = ctx.enter_context(tc.tile_pool(name="gps", bufs=1, space="PSUM"))

        # ---- resident SBUF tensors ----
        xt_sb = const.tile([128, KC_I, BC], F16)        # [p, kc, b] 16KB/p
        w1_sb = const.tile([128, E, KC_I, H1], F16)     # lhsT slices [128,128]
        w2_sb = const.tile([128, E, MC_H1, H2], F16)    # rhs slices [128,128]
        wg_sb = const.tile([128, KC_I, E * T], F16)
        # packed small constants: [b1 (16) | ident-f16-as-f32 (64) |
        # id16-f16-as-f32 (8) | bg (1)]
        co_sb = const.tile([128, 89], F32)
        b1_sb = co_sb[:, 0:16]
        id_sb = co_sb[:, 16:80].bitcast(F16)
        id16_sb = co_sb[:16, 80:88].bitcast(F16)
        bg_sb = co_sb[:16, 88:89]
        # towers staging for chunks 0..2: [b-part, t, global-btile, o]
        tow_sb = const.tile([128, T, (NCHUNK - 1) * NBT, H2], F16)
        # towers staging for the last chunk (PE diag combine): [o-part, t, b]
        outT2 = const.tile([128, T, CHUNK], F16)
        if has_b2:
            b2_sb = const.tile([1, E * H2], F16)
            on_sb = const.tile([1, 128], F16)
            nc.sync.dma_start(out=b2_sb[:], in_=t["b2r"])
            nc.sync.dma_start(out=on_sb[:], in_=t["ones1"])

        # x per chunk and W1 per expert-group; first-needed data (x chunk 0
        # kc0, W1 e=0 kc0) lands first.  HWDGE triggers cost ~625ns serial, so
        # keep the DMA count low.
        xt_r = t["xt"].rearrange("(kc p) b -> p kc b", p=128)
        w1_r = t["w1"].rearrange("e (kc p) m -> p e kc m", p=128)

        def load_x(c):
            sl = slice(c * CHUNK, (c + 1) * CHUNK)
            nc.sync.dma_start(out=xt_sb[:, :, sl], in_=xt_r[:, :, sl])

        def load_main():
            # demand order: L1 consumes W1 expert e at ~1.7us*e; xt chunk c at
            # ~14us*c; wg at ~1.7us; consts (b1) at first PSUM drain ~0.9us;
            # w2 at first L2 phase.  Boot DMAs are spread across SP/ACT/DVE
            # queues: each dma_start costs ~600ns on its issuing SEQ plus
            # ~630ns on the shared HWDGE, so fanning out the first few
            # shortens the critical path to the first matmul.
            load_x(0)
            nc.scalar.dma_start(out=w1_sb[:, 0], in_=w1_r[:, 0])
            nc.scalar.dma_start(out=co_sb[:], in_=t["consts"])
            nc.sync.dma_start(out=w1_sb[:, 1], in_=w1_r[:, 1])
            nc.scalar.dma_start(
                out=wg_sb[:], in_=t["wg"].rearrange("(kc p) g -> p kc g", p=128)
            )
            nc.sync.dma_start(out=w1_sb[:, 2:4], in_=w1_r[:, 2:4])
            nc.sync.dma_start(out=w1_sb[:, 4:8], in_=w1_r[:, 4:8])
            load_x(1)
            nc.sync.dma_start(
                out=w2_sb[:],
                in_=t["w2"].rearrange("e (kc p) o -> p e kc o", p=128),
            )
            load_x(2)
            load_x(3)

        def warmup():
            # PE warm-up: the HAM clock gate holds the PE at half clock until
            # ~3.4us of sustained activity.  The first real matmul cannot
            # start before its DMAs land (~3.8us), so spend that idle window
            # on dummy matmuls over a memset scratch tile -- by the time data
            # arrives the PE runs at full clock.  The dummy PSUM bank is
            # reclaimed by the first real accumulation (start=True).
            wu_sb = const.tile([128, 640], F16)
            nc.vector.memset(wu_sb[:], 0.0)
            wu_ps = hps_p.tile([128, CHUNK], F32, name="hp")
            for i in range(8):
                nc.tensor.matmul(
                    wu_ps[:],
                    wu_sb[:, 0:128],
                    wu_sb[:, 128:640],
                    start=True,
                    stop=True,
                )

        if not loop_loads:
            load_main()
            warmup()

        # dummies for part-disabled timing builds
        if "gates" not in P:
            wn_dummy = const.tile([128, NBT * E * T], F16)
            nc.vector.memset(wn_dummy[:], 0.125)
        if "l1" not in P:
            ht_dummy = const.tile([128, MC_H1, CHUNK], F16)
            nc.vector.memset(ht_dummy[:], 0.125)
        if "l2" not in P:
            eo_dummy = const.tile([128, E, H2], F16)
            nc.vector.memset(eo_dummy[:], 0.125)
        if "combine" not in P:
            nc.vector.memset(tow_sb[:], 0.0)
            nc.vector.memset(outT2[:], 0.0)

        def build_diags(wn):
            """Diag tiles for the last chunk's PE combine, built on the
            otherwise-idle GpSimd well before they are needed."""
            dgs = []
            for bt in range(NBT):
                dg = dg_p.tile([128, E * T, 128], F16)
                nc.gpsimd.tensor_mul(
                    dg[:],
                    id_sb[:].unsqueeze(1).broadcast_to([128, E * T, 128]),
                    wn[:, bt * E * T : (bt + 1) * E * T]
                    .unsqueeze(2)
                    .broadcast_to([128, E * T, 128]),
                )
                dgs.append(dg)
            return dgs

        def gates_wn(c):
            """Softmax gate weights for chunk c -> wn [128b, (bt, e, t)]."""
            cs = c * CHUNK
            if "gates" not in P:
                return wn_dummy
            lt_ps = gps_p.tile([16, CHUNK], F32, tag="g")
            for kc in range(KC_I):
                nc.tensor.matmul(
                    lt_ps[:],
                    wg_sb[:, kc, :],
                    xt_sb[:, kc, cs : cs + CHUNK],
                    start=(kc == 0),
                    stop=(kc == KC_I - 1),
                )
            # exp(logits + bg) while leaving PSUM, then transpose back
            ew = sm_p.tile([16, CHUNK], F16)
            nc.scalar.activation(ew[:], lt_ps[:], AF.Exp, bias=bg_sb[:, 0:1])
            gps = gps_p.tile([128, NBT, E * T], F16, tag="g")
            for bt in range(NBT):
                nc.tensor.transpose(
                    gps[:, bt, :],
                    ew[:, bt * 128 : (bt + 1) * 128],
                    id16_sb[:],
                )
            # sum over e (col index = bt*16 + e*2 + t)
            sums = sm_p.tile([128, NBT * T], F32)
            nc.vector.reduce_sum(
                sums[:].rearrange("p (bt t) -> p bt t", t=T),
                gps[:].rearrange("p bt (e t) -> p bt t e", e=E, t=T),
                axis=mybir.AxisListType.X,
            )
            recip = sm_p.tile([128, NBT * T], F32)
            nc.vector.reciprocal(recip[:], sums[:])
            wn = wn_p.tile([128, NBT * E * T], F16)
            nc.vector.tensor_mul(
                wn[:].rearrange("p (bt e t) -> p bt e t", e=E, t=T),
                gps[:].rearrange("p bt (e t) -> p bt e t", e=E, t=T),
                recip[:]
                .rearrange("p (bt t) -> p bt t", t=T)
                .unsqueeze(2)
                .broadcast_to([128, NBT, E, T]),
            )
            return wn

        rep_ctx = tc.For_i(0, reps, 1) if reps > 1 else None
        if rep_ctx is not None:
            ctx.enter_context(rep_ctx)
        if loop_loads:
            load_main()
        pending: dict = {}

        def emit_l1(c):
            """Layer 1 + relu for chunk c; chunk c+1's gates+diag are emitted
            mid-phase so their PE/ACT/DVE ops hide under L1."""
            cs = c * CHUNK
            hts = []
            for e in range(E):
                if "l1" in P:
                    ht = ht_p.tile([128, MC_H1, CHUNK], F16)
                    for mc in range(MC_H1):
                        hp = hps_p.tile([128, CHUNK], F32)
                        for kc in range(KC_I):
                            nc.tensor.matmul(
                                hp[:],
                                w1_sb[:, e, kc, mc * 128 : (mc + 1) * 128],
                                xt_sb[:, kc, cs : cs + CHUNK],
                                start=(kc == 0),
                                stop=(kc == KC_I - 1),
                            )
                        bcol = b1_sb[:, e * MC_H1 + mc : e * MC_H1 + mc + 1]
                        # all L1 drains on ACT: L1 then never waits on the
                        # DVE queue, which carries the gate-combine work
                        nc.scalar.activation(
                            ht[:, mc, :], hp[:], AF.Relu, bias=bcol
                        )
                    hts.append(ht)
                else:
                    hts.append(ht_dummy)
                if c == 0 and e == 0:
                    pending[0] = gates_wn(0)
                if e == 5 and c + 1 < NCHUNK:
                    pending[c + 1] = gates_wn(c + 1)
                    if c + 1 == NCHUNK - 1 and "combine" in P:
                        pending["dgs"] = build_diags(pending[c + 1])
            return hts

        def emit_l2(c, bt, hts):
            """Layer 2 + relu for b-tile bt of chunk c -> eo tile (fp16)."""
            bsl = slice(bt * 128, (bt + 1) * 128)
            if "l2" not in P:
                return eo_dummy
            eo = eo_p.tile([128, E, H2], F16)
            # two half-stacks: the relu drain of half 0 runs under half 1's
            # matmuls
            for half in range(2):
                eps = eps_p.tile([128, E // 2, H2], F32)
                for ei in range(E // 2):
                    e = half * (E // 2) + ei
                    for kc in range(MC_H1):
                        nc.tensor.matmul(
                            eps[:, ei, :],
                            hts[e][:, kc, bsl],
                            w2_sb[:, e, kc, :],
                            start=(kc == 0),
                            stop=(kc == MC_H1 - 1 and not has_b2),
                        )
                    if has_b2:
                        nc.tensor.matmul(
                            eps[:, ei, :],
                            on_sb[:1, :],
                            b2_sb[:1, e * H2 : (e + 1) * H2],
                            start=False,
                            stop=True,
                        )
                eo_sl = eo[:, half * (E // 2) : (half + 1) * (E // 2), :]
                if (bt + half) % 2 == 0 or EO_ALL_ACT:
                    nc.scalar.activation(
                        eo_sl.rearrange("p e o -> p (e o)"),
                        eps[:].rearrange("p e o -> p (e o)"),
                        AF.Relu,
                    )
                else:
                    nc.vector.tensor_scalar_max(
                        eo_sl.rearrange("p e o -> p (e o)"),
                        eps[:].rearrange("p e o -> p (e o)"),
                        0.0,
                    )
            return eo

        # flat output: [T, BC3*H2 (chunks 0..2, b-major) | H2*CHUNK (chunk 3,
        # o-major)] -- the two combine paths produce transposed layouts and
        # each gets the DMA-friendly one; the host stitches them.
        BC3 = (NCHUNK - 1) * CHUNK
        out1_r = t["out"][:, 0 : BC3 * H2].rearrange(
            "t (g p o) -> p t g o", p=128, o=H2
        )
        out2_r = t["out"][:, BC3 * H2 :].rearrange("t (o b) -> o t b", o=H2)

        def emit_combine(c, bt, eo, wn, dgs):
            """Gate combine for b-tile bt of chunk c.

            Chunks 0..2: towers[t] = sum_e eo_e * g_te on the DVE (g_te is a
            per-partition column; valid because g>0 commutes with the relu
            already applied to eo).  This runs off the PE and hides under the
            next chunk's L1.
            Last chunk: PE diag-matmul combine (towers.T accumulates
            sum_e eo_e.T @ diag(g_te) in PSUM) -- the PE is idle in the tail
            while the DVE is not, and the diag tiles were built on GpSimd
            long before.
            """
            if "combine" not in P:
                return
            if c < NCHUNK - 1:
                g = c * NBT + bt
                wn_v = wn[:].rearrange("p (bt e t) -> p bt e t", e=E, t=T)
                for t_ in range(T):
                    seo = seo_p.tile([128, E, H2], F16)
                    nc.vector.tensor_mul(
                        seo[:],
                        eo[:],
                        wn_v[:, bt, :, t_ : t_ + 1].broadcast_to([128, E, H2]),
                    )
                    # summing 8 fp16 terms into fp16 adds ~2^-11 relative
                    # error, well inside the 2e-2 budget
                    with nc.allow_low_precision(reason="fp16 gate combine"):
                        nc.vector.tensor_add(
                            seo[:, 0:4, :], seo[:, 0:4, :], seo[:, 4:8, :]
                        )
                        nc.vector.tensor_add(
                            seo[:, 0:2, :], seo[:, 0:2, :], seo[:, 2:4, :]
                        )
                        nc.vector.tensor_add(
                            tow_sb[:, t_, g, :], seo[:, 0, :], seo[:, 1, :]
                        )
                if bt == NBT - 1:
                    # out-DMA per chunk, one per task (DMA APs are limited to
                    # 3 dims)
                    gs = slice(c * NBT, (c + 1) * NBT)
                    for t_ in range(T):
                        nc.sync.dma_start(
                            out=out1_r[:, t_, gs, :], in_=tow_sb[:, t_, gs, :]
                        )
            else:
                tps = tps_p.tile([128, T, 128], F32)
                tflat = tps[:].rearrange("p t b -> p (t b)")
                for e in range(E):
                    nc.tensor.matmul(
                        tflat,
                        eo[:, e, :],
                        dgs[bt][:, e * T : (e + 1) * T, :].rearrange(
                            "p t b -> p (t b)"
                        ),
                        start=(e == 0),
                        stop=(e == E - 1),
                    )
                dst = outT2[:, :, bt * 128 : (bt + 1) * 128]
                if bt % 2 == 0:
                    nc.vector.tensor_copy(dst, tps[:])
                else:
                    nc.scalar.copy(dst, tps[:])
                if bt % 2 == 1:
                    # out-DMA per half-chunk; the final one goes on the idle
                    # SP queue (shorter tail) -- except in looped timing
                    # builds, where a final SP DMA would head-of-line-block
                    # the next iteration's input loads behind it
                    bs = slice((bt - 1) * 128, (bt + 1) * 128)
                    eng = (
                        nc.sync
                        if (bt == NBT - 1 and not loop_loads)
                        else nc.scalar
                    )
                    eng.dma_start(out=out2_r[:, :, bs], in_=outT2[:, :, bs])

        # per-chunk: L1, then L2 per b-tile; the gate-scaled combine runs on
        # the vector engines and is off the PE critical path entirely.
        for c in range(NCHUNK):
            hts = emit_l1(c)
            wn = pending.pop(c)
            if c < NCHUNK - 1:
                for bt in range(NBT):
                    emit_combine(c, bt, emit_l2(c, bt, hts), wn, None)
            else:
                # last chunk: PE combine is software-pipelined one b-tile
                # deep so its wait on the eo relu hides under L2 matmuls
                dgs = pending.pop("dgs", None)
                eos = {}
                for bt in range(NBT):
                    eos[bt] = emit_l2(c, bt, hts)
                    if bt >= 1:
                        emit_combine(c, bt - 1, eos.pop(bt - 1), wn, dgs)
                emit_combine(c, NBT - 1, eos.pop(NBT - 1), wn, dgs)


def _build(has_b2: bool, reps: int = 1, parts: str = "full"):
    nc = bacc.Bacc("TRN2", target_bir_lowering=False, debug=False)
    t = {
        "xt": nc.dram_tensor("xt", [I, BC], F16, kind="ExternalInput").ap(),
        "w1": nc.dram_tensor("w1", [E, I, H1], F16, kind="ExternalInput").ap(),
        "w2": nc.dram_tensor("w2", [E, H1, H2], F16, kind="ExternalInput").ap(),
        "wg": nc.dram_tensor("wg", [I, E * T], F16, kind="ExternalInput").ap(),
        "consts": nc.dram_tensor("consts", [128, 89], F32, kind="ExternalInput").ap(),
        "out": nc.dram_tensor("out", [T, BC * H2], F16, kind="ExternalOutput").ap(),
    }
    if has_b2:
        t["b2r"] = nc.dram_tensor("b2r", [1, E * H2], F16, kind="ExternalInput").ap()
        t["ones1"] = nc.dram_tensor("ones1", [1, 128], F16, kind="ExternalInput").ap()
    _emit(nc, t, has_b2, reps=reps, parts=parts)
    nc.compile()
    return nc


def _get_nc(has_b2: bool):
    key = ("nc", has_b2)
    if key not in _CACHE:
        _CACHE[key] = _build(has_b2)
    return _CACHE[key]


def _host_consts(b1=None, bg=None):
    """Packed consts: [b1 (16) | ident-f16 (64) | id16-f16 (8) | bg (1)]."""
    coh = np.zeros((128, 89), np.float32)
    if b1 is not None:
        coh[:, 0:16] = np.broadcast_to(
            b1.reshape(E, MC_H1, 128).transpose(2, 0, 1), (128, E, MC_H1)
        ).reshape(128, E * MC_H1)
    coh[:, 16:80] = np.ascontiguousarray(np.eye(128, dtype=np.float16)).view(
        np.float32
    )
    coh[:16, 80:88] = np.ascontiguousarray(np.eye(16, dtype=np.float16)).view(
        np.float32
    )
    if bg is not None:
        coh[:16, 88] = bg.T.reshape(E * T)
    return coh


def _host_prep(x, W1, b1, W2, b2, Wg, bg, has_b2):
    w1h = np.ascontiguousarray(W1.astype(np.float16))
    w2h = np.ascontiguousarray(W2.astype(np.float16))
    wgh = np.ascontiguousarray(
        Wg.transpose(1, 2, 0).reshape(I, E * T).astype(np.float16)
    )
    xth = np.ascontiguousarray(x.T.astype(np.float16))
    coh = _host_consts(b1, bg)
    in_maps = []
    for core in range(NCORES):
        m = {
            "xt": np.ascontiguousarray(xth[:, core * BC : (core + 1) * BC]),
            "w1": w1h,
            "w2": w2h,
            "wg": wgh,
            "consts": coh,
        }
        if has_b2:
            m["b2r"] = b2.astype(np.float16).reshape(1, E * H2)
            m["ones1"] = np.ones((1, 128), np.float16)
        in_maps.append(m)
    return in_maps


def _build_runner(nc):
    """Cached replica of bass2jax.run_bass_via_pjrt's multi-core path: the
    jitted shard_map callable is built once and reused across kernel() calls."""
    import jax
    from jax.experimental.shard_map import shard_map
    from jax.sharding import Mesh, PartitionSpec

    from concourse import bass2jax, mybir as mb

    bass2jax.install_neuronx_cc_hook()
    partition_name = (
        nc.partition_id_tensor.name if nc.partition_id_tensor else None
    )
    in_names, out_names, out_avals, zero_shapes = [], [], [], []
    for alloc in nc.m.functions[0].allocations:
        if not isinstance(mb.MemoryLocationSet, type) or not isinstance(
            alloc, mb.MemoryLocationSet
        ):
            continue
        name = alloc.memorylocations[0].name
        if alloc.kind == "ExternalInput":
            if name != partition_name:
                in_names.append(name)
        elif alloc.kind == "ExternalOutput":
            shape = tuple(alloc.tensor_shape)
            dtype = mb.dt.np(alloc.dtype)
            out_names.append(name)
            out_avals.append(jax.core.ShapedArray(shape, dtype))
            zero_shapes.append((shape, dtype))
    n_params = len(in_names)
    n_outs = len(out_avals)
    all_in_names = list(in_names) + list(out_names)
    if partition_name is not None:
        all_in_names.append(partition_name)
    donate = tuple(range(n_params, n_params + n_outs))

    def _body(*args):
        operands = list(args)
        if partition_name is not None:
            operands.append(bass2jax.partition_id_tensor())
        outs = bass2jax._bass_exec_p.bind(
            *operands,
            out_avals=tuple(out_avals),
            in_names=tuple(all_in_names),
            out_names=tuple(out_names),
            lowering_input_output_aliases=(),
            sim_require_finite=True,
            sim_require_nnan=True,
            nc=nc,
        )
        return tuple(outs)

    devices = jax.devices()[:NCORES]
    mesh = Mesh(np.asarray(devices), ("core",))
    in_specs = (PartitionSpec("core"),) * (n_params + n_outs)
    out_specs = (PartitionSpec("core"),) * n_outs
    sharded = jax.jit(
        shard_map(
            _body, mesh=mesh, in_specs=in_specs, out_specs=out_specs,
            check_rep=False,
        ),
        donate_argnums=donate,
        keep_unused=True,
    )

    def run(in_maps):
        concat_in = [
            np.concatenate([np.asarray(m[name]) for m in in_maps], axis=0)
            for name in in_names
        ]
        concat_zeros = [
            np.zeros((NCORES * s[0], *s[1:]), d) for s, d in zero_shapes
        ]
        out_arrs = sharded(*concat_in, *concat_zeros)
        return [
            {
                name: np.asarray(out_arrs[i]).reshape(
                    NCORES, *zero_shapes[i][0]
                )[c]
                for i, name in enumerate(out_names)
            }
            for c in range(NCORES)
        ]

    return run


def kernel(x, W1, b1, W2, b2, Wg, bg):
    x = np.asarray(x, np.float32)
    W1 = np.asarray(W1, np.float32)
    b1 = np.asarray(b1, np.float32)
    W2 = np.asarray(W2, np.float32)
    b2 = np.asarray(b2, np.float32)
    Wg = np.asarray(Wg, np.float32)
    bg = np.asarray(bg, np.float32)

    has_b2 = bool(np.any(b2))
    nc = _get_nc(has_b2)
    in_maps = _host_prep(x, W1, b1, W2, b2, Wg, bg, has_b2)

    key = ("runner", has_b2)
    try:
        if key not in _CACHE:
            _CACHE[key] = _build_runner(nc)
        results = _CACHE[key](in_maps)
    except Exception:
        _CACHE.pop(key, None)
        results = run_bass_kernel_spmd(
            nc, in_maps, core_ids=list(range(NCORES))
        ).results
    # gather: per-core flat out [T, BC*H2] fp16 = [chunks 0-2 b-major |
    # chunk 3 o-major] -> [T, B, H2] fp32
    bc3 = (NCHUNK - 1) * CHUNK
    outs = []
    for r in results:
        flat = r["out"]
        p1 = flat[:, : bc3 * H2].reshape(T, bc3, H2)
        p2 = flat[:, bc3 * H2 :].reshape(T, H2, CHUNK).transpose(0, 2, 1)
        outs.append(np.concatenate([p1, p2], axis=1))
    return np.concatenate(outs, axis=1).astype(np.float32)

